# revision 2
# baseline (speedup 1.0000x reference)
# Tensor-parallel fused attention kernel for TRN2, 8 cores.
# Core r owns heads {2r, 2r+1}. Inputs per core:
#   x     [2*S, 1024] f32  (replicated; rows = b*S + s)
#   wqkv  [1024, 512] f32  (columns: q_h1|q_h2|k_h1|k_h2|v_h1|v_h2|g_h1|g_h2, 64 each)
#   wout  [1024, 128] f32  (w_out columns 128r:128r+128)
#   freqs [S, 32] f32      (replicated)
#   lnp   [8, 64] f32      ([qn_w, qn_w, kn_w, kn_w, qn_b, qn_b, kn_b, kn_b])
# Output per core:
#   out   [128, 2*S] f32   (out^T slice: rows = w_out columns owned by this core)
import math

import concourse.bass as bass
import concourse.mybir as mybir
from concourse import bacc, tile

F32 = mybir.dt.float32
F16 = mybir.dt.float16
AF = mybir.ActivationFunctionType
ALU = mybir.AluOpType

DIM = 1024
HD = 64          # head dim
EPS = 1e-5


def build(S: int, n_cores: int = 8):
    TB = S // 128            # t-tiles per batch
    TT = 2 * TB              # total t-tiles
    JT = S // 64             # 64-wide kj tiles per batch
    QW = min(512, S)         # qi tile width
    QT = S // QW             # qi tiles per batch
    J_PER_EXP = 2 if S >= 128 else 1   # J tiles fused into one exp op (psum_s width J_PER_EXP*512)
    SW = QW                  # matmul N for S^T matmuls

    nc = bacc.Bacc("TRN2", target_bir_lowering=False, debug=False, num_devices=n_cores)

    X = nc.dram_tensor("x", [2 * S, DIM], F32, kind="ExternalInput")
    WQKV = nc.dram_tensor("wqkv", [DIM, 512], F32, kind="ExternalInput")
    WOUT = nc.dram_tensor("wout", [DIM, 128], F32, kind="ExternalInput")
    FREQS = nc.dram_tensor("freqs", [S, 32], F32, kind="ExternalInput")
    LNP = nc.dram_tensor("lnp", [8, HD], F32, kind="ExternalInput")
    SEL2 = nc.dram_tensor("sel2", [2, 128], F32, kind="ExternalInput")
    OUT = nc.dram_tensor("out", [128, 2 * S], F32, kind="ExternalOutput")

    ag_in = nc.dram_tensor("ag_in", [128, 2 * S], F16)
    ag_out = nc.dram_tensor("ag_out", [8 * 128, 2 * S], F16, addr_space="Shared")

    with tile.TileContext(nc) as tc:
        with (
            tc.tile_pool(name="persist", bufs=1) as pp,
            tc.tile_pool(name="work", bufs=3) as wp,
            tc.tile_pool(name="small", bufs=4) as sp,
        ):
            # ---- constants & weights ----
            w16 = pp.tile([128, 8, 512], F16, tag="w16")          # qkv weights (k-tiles)
            w16o = pp.tile([128, 8, 128], F16, tag="w16o")        # out-proj weights
            for c in range(8):
                w32 = wp.tile([128, 512], F32, tag="wld")
                nc.sync.dma_start(w32[:], WQKV[c * 128:(c + 1) * 128, :])
                nc.vector.tensor_copy(w16[:, c, :], w32[:])
                wo32 = wp.tile([128, 128], F32, tag="wold")
                nc.sync.dma_start(wo32[:], WOUT[c * 128:(c + 1) * 128, :])
                nc.vector.tensor_copy(w16o[:, c, :], wo32[:])

            # ln params broadcast [128, 512] (w's 0:256, b's 256:512)
            lnp1 = sp.tile([1, 512], F32, tag="lnp1")
            nc.sync.dma_start(lnp1[:], LNP.ap().rearrange("a b -> (a b)").unsqueeze(0))
            ones1 = sp.tile([1, 128], F32, tag="ones1")
            nc.vector.memset(ones1[:], 1.0)
            with tc.tile_pool(name="pbc", bufs=1, space="PSUM") as pbc:
                lnb_ps = pbc.tile([128, 512], F32)
                nc.tensor.matmul(lnb_ps[:], ones1[:], lnp1[:], start=True, stop=True)
                lnwb = pp.tile([128, 512], F32, tag="lnwb")
                nc.scalar.copy(lnwb[:], lnb_ps[:])

            # cos/sin tables [128, TB*32]
            ftile = sp.tile([128, TB * 32], F32, tag="ftile")
            nc.sync.dma_start(
                ftile[:].rearrange("p (a c) -> p a c", c=32),
                bass.AP(FREQS.ap().tensor, 0, [[32, 128], [128 * 32, TB], [1, 32]]),
            )
            sin_t = pp.tile([128, TB * 32], F32, tag="sin_t")
            cos_t = pp.tile([128, TB * 32], F32, tag="cos_t")
            halfpi = sp.tile([128, 1], F32, tag="halfpi")
            nc.vector.memset(halfpi[:], math.pi / 2)
            epsc = pp.tile([128, 1], F32, tag="epsc")
            nc.vector.memset(epsc[:], EPS)
            nc.scalar.activation(sin_t[:], ftile[:], AF.Sin)
            nc.scalar.activation(cos_t[:], ftile[:], AF.Sin, bias=halfpi[:])

            # ones_bd [128,2] f16 for denominator matmul; sel2 [2,128] f32 for recip bcast
            ones_bd = pp.tile([128, 2], F16, tag="ones_bd")
            nc.vector.memset(ones_bd[:], 0.0)
            nc.vector.memset(ones_bd[0:64, 0:1], 1.0)
            nc.vector.memset(ones_bd[64:128, 1:2], 1.0)
            sel2 = pp.tile([2, 128], F32, tag="sel2")
            nc.sync.dma_start(sel2[:], SEL2[:])

            # ---- persistent activations ----
            qT = pp.tile([128, TT, 128], F16, tag="qT")
            kT = pp.tile([128, TT, 128], F16, tag="kT")
            gT = pp.tile([128, TT, 128], F16, tag="gT")
            v_nat = pp.tile([128, TT, 128], F16, tag="v_nat")
            og = pp.tile([128, 2 * S], F16, tag="og")
            k_bd = pp.tile([128, JT, 128], F16, tag="k_bd")
            v_bd = pp.tile([128, JT, 128], F16, tag="v_bd")
            nc.vector.memset(k_bd[:], 0.0)
            nc.vector.memset(v_bd[:], 0.0)

            # ---- phase 1: qkv matmul + LN + rope + transposes ----
            with tc.tile_pool(name="ps_qkv", bufs=2, space="PSUM") as pq:
                for T in range(TT):
                    xt32 = wp.tile([128, DIM], F32, tag="xt32")
                    nc.sync.dma_start(xt32[:], X[T * 128:(T + 1) * 128, :])
                    xt16 = wp.tile([128, DIM], F16, tag="xt16")
                    nc.vector.tensor_copy(xt16[:], xt32[:])
                    xT16 = wp.tile([128, 8, 128], F16, tag="xT16")
                    for c in range(8):
                        nc.sync.dma_start_transpose(
                            xT16[:, c, :], xt16[:, c * 128:(c + 1) * 128])

                    psq = pq.tile([128, 512], F32, tag="psq")
                    for c in range(8):
                        nc.tensor.matmul(psq[:], xT16[:, c, :], w16[:, c, :],
                                         start=(c == 0), stop=(c == 7))

                    # LN on q,k segments ([128, 4, 64] view of psq[:, 0:256])
                    qk_ps = psq[:, 0:256].rearrange("p (a b) -> p a b", b=HD)
                    negm = sp.tile([128, 4], F32, tag="negm")
                    nc.vector.tensor_reduce(negm[:], qk_ps, mybir.AxisListType.X, ALU.add)
                    nc.vector.tensor_scalar_mul(negm[:], negm[:], -1.0 / HD)
                    xc = wp.tile([128, 256], F32, tag="xc")
                    xc3 = xc[:].rearrange("p (a b) -> p a b", b=HD)
                    nc.vector.tensor_tensor(
                        xc3, qk_ps, negm[:].unsqueeze(2).broadcast_to([128, 4, HD]), ALU.add)
                    sq = wp.tile([128, 256], F32, tag="sq")
                    nc.scalar.activation(sq[:], xc[:], AF.Square)
                    ssq = sp.tile([128, 4], F32, tag="ssq")
                    nc.vector.tensor_reduce(
                        ssq[:], sq[:].rearrange("p (a b) -> p a b", b=HD),
                        mybir.AxisListType.X, ALU.add)
                    std = sp.tile([128, 4], F32, tag="std")
                    nc.scalar.activation(std[:], ssq[:], AF.Sqrt, scale=1.0 / HD, bias=epsc[:])
                    rstd = sp.tile([128, 4], F32, tag="rstd")
                    nc.vector.reciprocal(rstd[:], std[:])
                    xn = wp.tile([128, 256], F32, tag="xn")
                    xn3 = xn[:].rearrange("p (a b) -> p a b", b=HD)
                    nc.vector.tensor_tensor(
                        xn3, xc3, rstd[:].unsqueeze(2).broadcast_to([128, 4, HD]), ALU.mult)
                    nc.vector.tensor_tensor(xn[:], xn[:], lnwb[:, 0:256], ALU.mult)
                    nc.vector.tensor_tensor(xn[:], xn[:], lnwb[:, 256:512], ALU.add)

                    # rope: pairs (even, odd) along free dim within each 64-seg
                    st = T % TB
                    cosb = cos_t[:, st * 32:(st + 1) * 32].unsqueeze(1).broadcast_to([128, 4, 32])
                    sinb = sin_t[:, st * 32:(st + 1) * 32].unsqueeze(1).broadcast_to([128, 4, 32])
                    xe = bass.AP(xn.tensor, xn[:].offset,
                                 [xn[:].ap[0], [HD, 4], [2, 32]])
                    xo = bass.AP(xn.tensor, xn[:].offset + 1,
                                 [xn[:].ap[0], [HD, 4], [2, 32]])
                    t1 = wp.tile([128, 128], F32, tag="t1")
                    t2 = wp.tile([128, 128], F32, tag="t2")
                    t13 = t1[:].rearrange("p (a b) -> p a b", b=32)
                    t23 = t2[:].rearrange("p (a b) -> p a b", b=32)
                    qk16 = wp.tile([128, 256], F16, tag="qk16")
                    qke = bass.AP(qk16.tensor, qk16[:].offset,
                                  [qk16[:].ap[0], [HD, 4], [2, 32]])
                    qko = bass.AP(qk16.tensor, qk16[:].offset + 1,
                                  [qk16[:].ap[0], [HD, 4], [2, 32]])
                    nc.vector.tensor_tensor(t13, xe, cosb, ALU.mult)
                    nc.vector.tensor_tensor(t23, xo, sinb, ALU.mult)
                    nc.vector.tensor_tensor(qke, t13, t23, ALU.subtract)
                    nc.vector.tensor_tensor(t13, xe, sinb, ALU.mult)
                    nc.vector.tensor_tensor(t23, xo, cosb, ALU.mult)
                    nc.vector.tensor_tensor(qko, t13, t23, ALU.add)

                    # v (cast) and gate (sigmoid) + transposes
                    nc.scalar.copy(v_nat[:, T, :], psq[:, 256:384])
                    gs16 = wp.tile([128, 128], F16, tag="gs16")
                    nc.scalar.activation(gs16[:], psq[:, 384:512], AF.Sigmoid)
                    nc.sync.dma_start_transpose(gT[:, T, :], gs16[:])
                    nc.sync.dma_start_transpose(qT[:, T, :], qk16[:, 0:128])
                    nc.sync.dma_start_transpose(kT[:, T, :], qk16[:, 128:256])

            # ---- phase 2: attention per batch ----
            with (
                tc.tile_pool(name="ps_s", bufs=2, space="PSUM") as ps_s,
                tc.tile_pool(name="ps_o", bufs=1, space="PSUM") as ps_o,
                tc.tile_pool(name="ps_d", bufs=1, space="PSUM") as ps_d,
                tc.tile_pool(name="ps_r", bufs=1, space="PSUM") as ps_r,
            ):
                for b in range(2):
                    # build block-diagonal k / v tiles for this batch
                    for J in range(JT):
                        Tl = b * TB + J // 2
                        off = 64 * (J % 2)
                        nc.vector.tensor_copy(k_bd[0:64, J, 0:64],
                                              kT[0:64, Tl, off:off + 64])
                        nc.vector.tensor_copy(k_bd[64:128, J, 64:128],
                                              kT[64:128, Tl, off:off + 64])
                        nc.vector.tensor_copy(v_bd[0:64, J, 0:64],
                                              v_nat[off:off + 64, Tl, 0:64])
                        nc.vector.tensor_copy(v_bd[64:128, J, 64:128],
                                              v_nat[off:off + 64, Tl, 64:128])

                    for Q in range(QT):
                        qslice = qT[:, b * TB + Q * (QW // 128):
                                    b * TB + (Q + 1) * (QW // 128), :]
                        qs2 = qslice.rearrange("p a b -> p (a b)")
                        po = ps_o.tile([128, QW], F32, tag="po")
                        pd = ps_d.tile([2, QW], F32, tag="pd")
                        for Jb in range(JT // J_PER_EXP):
                            ps = ps_s.tile([128, J_PER_EXP * SW], F32, tag="ps")
                            for jj in range(J_PER_EXP):
                                J = Jb * J_PER_EXP + jj
                                nc.tensor.matmul(ps[:, jj * SW:(jj + 1) * SW],
                                                 k_bd[:, J, :], qs2,
                                                 start=True, stop=True)
                            es = wp.tile([128, J_PER_EXP * SW], F16, tag="es")
                            nc.scalar.activation(es[:], ps[:], AF.Exp, scale=0.125)
                            for jj in range(J_PER_EXP):
                                J = Jb * J_PER_EXP + jj
                                esj = es[:, jj * SW:(jj + 1) * SW]
                                nc.tensor.matmul(po[:], v_bd[:, J, :], esj,
                                                 start=(J == 0), stop=(J == JT - 1),
                                                 skip_group_check=True)
                                nc.tensor.matmul(pd[:], ones_bd[:], esj,
                                                 start=(J == 0), stop=(J == JT - 1),
                                                 skip_group_check=True)
                        rd = sp.tile([2, QW], F32, tag="rd")
                        nc.vector.reciprocal(rd[:], pd[:])
                        pr = ps_r.tile([128, QW], F32, tag="pr")
                        nc.tensor.matmul(pr[:], sel2[:], rd[:], start=True, stop=True)
                        r32 = wp.tile([128, QW], F32, tag="r32")
                        nc.scalar.copy(r32[:], pr[:])
                        on = wp.tile([128, QW], F32, tag="on")
                        nc.vector.tensor_tensor(on[:], po[:], r32[:], ALU.mult)
                        gslice = gT[:, b * TB + Q * (QW // 128):
                                    b * TB + (Q + 1) * (QW // 128), :]
                        gq = b * QT + Q
                        nc.vector.tensor_tensor(
                            og[:, gq * QW:(gq + 1) * QW],
                            on[:], gslice.rearrange("p a b -> p (a b)"), ALU.mult)

            # ---- phase 3: all-gather + out projection ----
            nc.sync.dma_start(ag_in.ap(), og[:])
            nc.gpsimd.collective_compute(
                "AllGather", ALU.bypass,
                replica_groups=[list(range(n_cores))],
                ins=[ag_in.ap()], outs=[ag_out.ap()],
            )
            with tc.tile_pool(name="ps_ot", bufs=2, space="PSUM") as ps_ot:
                n_tt = (2 * S) // 512
                for TTi in range(n_tt):
                    pot = ps_ot.tile([128, 512], F32, tag="pot")
                    for c in range(8):
                        ogf = wp.tile([128, 512], F16, tag="ogf")
                        nc.sync.dma_start(
                            ogf[:], ag_out[c * 128:(c + 1) * 128,
                                           TTi * 512:(TTi + 1) * 512])
                        nc.tensor.matmul(pot[:], w16o[:, c, :], ogf[:],
                                         start=(c == 0), stop=(c == 7))
                    ot32 = wp.tile([128, 512], F32, tag="ot32")
                    nc.scalar.copy(ot32[:], pot[:])
                    nc.sync.dma_start(OUT[:, TTi * 512:(TTi + 1) * 512], ot32[:])

    nc.compile()
    return nc


def shard_inputs(x, freqs, w_qkv, w_out, qn_w, qn_b, kn_w, kn_b, n_cores=8):
    """Full inputs -> list of per-core input dicts."""
    import numpy as np
    B, S, _ = x.shape
    x2 = np.ascontiguousarray(x.reshape(2 * S, DIM), dtype=np.float32)
    lnp_base = np.stack([qn_w, qn_w, kn_w, kn_w, qn_b, qn_b, kn_b, kn_b]).astype(np.float32)
    sel2c = np.zeros((2, 128), np.float32)
    sel2c[0, 0:64] = 1.0
    sel2c[1, 64:128] = 1.0
    maps = []
    for r in range(n_cores):
        cols = []
        for sec in range(4):
            c0 = sec * DIM + 128 * r
            cols.append(w_qkv[:, c0:c0 + 128])
        wq = np.ascontiguousarray(np.concatenate(cols, axis=1), dtype=np.float32)
        wo = np.ascontiguousarray(w_out[:, 128 * r:128 * (r + 1)], dtype=np.float32)
        maps.append({
            "x": x2, "wqkv": wq, "wout": wo,
            "freqs": np.ascontiguousarray(freqs, dtype=np.float32),
            "lnp": lnp_base, "sel2": sel2c,
        })
    return maps


def unshard_output(results, S):
    """list of per-core {'out': [128, 2S]} -> [2, S, 1024]."""
    import numpy as np
    outT = np.concatenate([r["out"] for r in results], axis=0)  # [1024, 2S]
    return np.ascontiguousarray(outT.T).reshape(2, S, DIM)


_NC_CACHE = {}


def _get_nc(S):
    if S not in _NC_CACHE:
        _NC_CACHE[S] = build(S)
    return _NC_CACHE[S]


def kernel(x, freqs, w_qkv, w_out, qn_w, qn_b, kn_w, kn_b):
    """Full-input entrypoint: shards across 8 neuron cores, runs, gathers."""
    import numpy as np
    from concourse.bass_utils import run_bass_kernel_spmd

    B, S, _ = x.shape
    nc = _get_nc(S)
    maps = shard_inputs(x, freqs, w_qkv, w_out, qn_w, qn_b, kn_w, kn_b)
    res = run_bass_kernel_spmd(nc, maps, list(range(8)))
    return unshard_output(res.results, S)
